# revision 1
# baseline (speedup 1.0000x reference)
"""Trainium2 Bass kernel for nn_InterFusion (dense transformer block, S=2).

Strategy (v2):
  - Pure data parallel: batch (8192) split across 8 NeuronCores; weights
    replicated, fp16 on-chip (PSUM accumulation stays fp32).
  - Feature-major layout: activations live as X^T tiles [feat(128) x tokens].
  - Both LayerNorms are folded algebraically into the matmuls so the heavy
    projections run on RAW (un-normalized) activations with no dependency on
    the LN statistics:
      LN1:  q = r*(Wq@x) - r*m*rowsum(Wq)  (same for k,v).  The softmax needs
            only score DIFFERENCES over t, so per-head corrections reduce to
            D[h,s] = r^2*(S~d[h,s] - m*Ad[h]) with S~d from q*(K0-K1) products
            and Ad from a cq-masked head-sum of (K0-K1); 2-way softmax is a
            single Sigmoid.  V-side corrections collapse into a rank-1 (K=1)
            matmul accumulated straight into the O-projection PSUM.
      LN2:  pass 2 scales o1 by r2 (per-column) before the FFN W1 matmul and
            accumulates the -r2*m2*rowsum(W1) correction as a K=1 matmul.
  - Two passes over DRAM: pass1 = LN1+QKV+attn+O-proj+residual (+LN2 stats),
    spilling out1 (fp16); pass2 = per-position FFN + final residual.
    FFN weights for position 0 prefetch during pass 1, position 1 during
    pass 2a, so the PE never waits on weight DMA.
"""

import sys

for _p in ("/opt/trn_rl_repo", "/root/.axon_site/_ro/trn_rl_repo"):
    if _p not in sys.path:
        sys.path.append(_p)

import ml_dtypes
import numpy as np

import concourse.bass as bass
import concourse.bacc as bacc
import concourse.tile as tile
from concourse import mybir
from concourse import bass_utils

F32 = mybir.dt.float32
F16 = mybir.dt.float16
F8 = mybir.dt.float8e4
AT = mybir.ActivationFunctionType
OP = mybir.AluOpType

DEBUG = False

E = 1024
S = 2
B = 8192
NCORES = 8
ROWS = B // NCORES          # 1024 rows per core
P = 128
NCH = E // P                # 8 feature chunks
HID = 2 * E
NHID = HID // P             # 16 hidden chunks
EPS = 1e-5

SQ = 128.0                  # fp8 pre-scale for Wq (incl /8 fold)
SK = 32.0                   # fp8 pre-scale for Wk
SV = 32.0                   # fp8 pre-scale for Wv
SW1 = 32.0                  # fp8 pre-scale for W1
SW2 = 32.0                  # fp8 pre-scale for W2

H1 = 128                    # pass-1: rows per tile
T1 = 2 * H1                 # 256 cols (s-major: s*128 + j)
NT = ROWS // H1             # 8 tiles
T2 = 256                    # pass-2: rows per tile (single position)
NU = ROWS // T2             # 4 tiles per position


def _ap(handle_ap, offset, dims):
    return bass.AP(tensor=handle_ap.tensor, offset=handle_ap.offset + offset,
                   ap=[list(d) for d in dims])


def _tap(t, offset, dims):
    """AP on SBUF/PSUM tile t with explicit free dims; partition dim kept."""
    base = t[:]
    pdim = base.ap[0]
    return bass.AP(tensor=base.tensor, offset=base.offset + offset,
                   ap=[list(pdim)] + [list(d) for d in dims])


def build(nc):
    # ---------------- DRAM I/O ----------------
    xprep = nc.dram_tensor("xprep", [NT * NCH * P, T1], F16,
                           kind="ExternalInput")
    xhi8 = nc.dram_tensor("xhi8", [NT * NCH * P, T1], F8,
                          kind="ExternalInput")
    xlo8 = nc.dram_tensor("xlo8", [NT * NCH * P, T1], F8,
                          kind="ExternalInput")
    dxhi8 = nc.dram_tensor("dxhi8", [NT * NCH * P, H1], F8,
                           kind="ExternalInput")
    dxlo8 = nc.dram_tensor("dxlo8", [NT * NCH * P, H1], F8,
                           kind="ExternalInput")
    wk8 = [nc.dram_tensor(f"wk8{i}", [P, NCH * E], F8, kind="ExternalInput")
           for i in range(2)]
    wq8 = [nc.dram_tensor(f"wq8{i}", [P, NCH * E], F8, kind="ExternalInput")
           for i in range(2)]
    wv8 = [nc.dram_tensor(f"wv8{i}", [P, NCH * E], F8, kind="ExternalInput")
           for i in range(2)]
    wobig = nc.dram_tensor("wobig", [P, NCH * E], F16, kind="ExternalInput")
    w18 = [[nc.dram_tensor(f"w18{f}{i}", [P, NCH * HID], F8,
                           kind="ExternalInput") for i in range(2)]
           for f in range(2)]
    w28 = [[nc.dram_tensor(f"w28{f}{i}", [P, NHID * E], F8,
                           kind="ExternalInput") for i in range(2)]
           for f in range(2)]
    b1f = [nc.dram_tensor(f"b1f{f}", [P, NHID], F32, kind="ExternalInput")
           for f in range(2)]
    b2f = [nc.dram_tensor(f"b2f{f}", [P, NCH], F32, kind="ExternalInput")
           for f in range(2)]
    hsel = nc.dram_tensor("hsel", [P, 16], F16, kind="ExternalInput")
    hselT = nc.dram_tensor("hselT", [16, P], F16, kind="ExternalInput")
    cqsel = nc.dram_tensor("cqsel", [P, NCH * 16], F16, kind="ExternalInput")
    cov = nc.dram_tensor("cov", [1, E], F16, kind="ExternalInput")
    onesP = nc.dram_tensor("onesP", [P, 1], F16, kind="ExternalInput")
    onesR = nc.dram_tensor("onesR", [1, P], F16, kind="ExternalInput")
    ones16 = nc.dram_tensor("ones16", [1, 16], F16, kind="ExternalInput")
    ones8 = nc.dram_tensor("ones8", [P, 32], F8, kind="ExternalInput")

    o1T = nc.dram_tensor("o1T", [2 * NCH * P, ROWS], F16, kind="Internal")
    if DEBUG:
        dbg = {nm: nc.dram_tensor(f"dbg_{nm}", shp, dt, kind="ExternalOutput")
               for nm, shp, dt in [
                   ("stc", [1, 2 * T1], F32), ("rf2", [1, T1], F16),
                   ("dkt", [P, E], F16), ("d2", [16, T1], F32),
                   ("p_r", [16, T1], F16), ("z", [P, NCH * T1], F16),
                   ("nrmu2", [1, T1], F16), ("v1r", [P, E], F16),
                   ("rmu", [1, H1], F32), ("mmsq", [1, T1], F32),
                   ("rr", [1, H1], F32)]}
    outT = nc.dram_tensor("outT", [2 * NU * NCH * P, T2], F32,
                          kind="ExternalOutput")

    MM = nc.tensor.matmul

    def dma(out, in_):
        nc.sync.dma_start(out=out, in_=in_)

    with tile.TileContext(nc) as tc:
        from contextlib import ExitStack

        with ExitStack() as outer:
            cpool = outer.enter_context(tc.tile_pool(name="c", bufs=1))
            wff0 = outer.enter_context(tc.tile_pool(name="wff0", bufs=1))

            # constants
            hsel_sb = cpool.tile([P, 16], F16, tag="hsel", name="hsel")
            hselT_sb = cpool.tile([16, P], F16, tag="hselT", name="hselT")
            cqsel_sb = cpool.tile([P, NCH * 16], F16, tag="cqsel",
                                  name="cqsel")
            cov_sb = cpool.tile([1, E], F16, tag="cov", name="cov")
            onesP_sb = cpool.tile([P, 1], F16, tag="onesP", name="onesP")
            onesR_sb = cpool.tile([1, P], F16, tag="onesR", name="onesR")
            ones16_sb = cpool.tile([1, 16], F16, tag="ones16", name="ones16")
            ones8_sb = cpool.tile([P, 32], F8, tag="ones8", name="ones8")
            b1_sb = [cpool.tile([P, NHID], F32, tag=f"b1{f}", name=f"b1{f}")
                     for f in range(2)]
            b2_sb = [cpool.tile([P, NCH], F32, tag=f"b2{f}", name=f"b2{f}")
                     for f in range(2)]
            eps_sb = cpool.tile([1, 1], F32, tag="eps", name="eps")
            nc.vector.memset(eps_sb[:], EPS)
            # persistent LN2 rows (written pass 1, read pass 2)
            r2row = cpool.tile([1, ROWS], F16, tag="r2row", name="r2row")
            nr2m2 = cpool.tile([1, ROWS], F16, tag="nr2m2", name="nr2m2")

            dma(onesP_sb[:], onesP.ap())
            dma(ones8_sb[:], ones8.ap())

            # FFN position-0 weights (prefetched during pass 1)
            w1_sb0 = [wff0.tile([P, NCH * HID], F8, tag=f"w1{i}",
                                name=f"w1s0{i}") for i in range(2)]
            w2_sb0 = [wff0.tile([P, NHID * E], F8, tag=f"w2{i}",
                                name=f"w2s0{i}") for i in range(2)]

            # =================== PASS 1 ===================
            with ExitStack() as p1:
                wp = p1.enter_context(tc.tile_pool(name="p1w", bufs=1))
                xp = p1.enter_context(tc.tile_pool(name="p1x", bufs=2))
                sqp = p1.enter_context(tc.tile_pool(name="p1sq", bufs=2))
                sq8p = p1.enter_context(tc.tile_pool(name="p1sq8", bufs=1))
                dkp = p1.enter_context(tc.tile_pool(name="p1dk", bufs=1))
                prp = p1.enter_context(tc.tile_pool(name="p1pr", bufs=1))
                zp = p1.enter_context(tc.tile_pool(name="p1z", bufs=1))
                ztp = p1.enter_context(tc.tile_pool(name="p1zt", bufs=2))
                o1p = p1.enter_context(tc.tile_pool(name="p1o1", bufs=2))
                rwp = p1.enter_context(tc.tile_pool(name="p1row", bufs=1))
                rvp = p1.enter_context(tc.tile_pool(name="p1rv", bufs=2))
                smp = p1.enter_context(tc.tile_pool(name="p1sm", bufs=1))
                pp = p1.enter_context(
                    tc.tile_pool(name="p1mm", bufs=4, space="PSUM"))
                ps_st = p1.enter_context(
                    tc.tile_pool(name="p1st", bufs=1, space="PSUM"))
                ps_a = p1.enter_context(
                    tc.tile_pool(name="p1a", bufs=1, space="PSUM"))
                ps_b = p1.enter_context(
                    tc.tile_pool(name="p1b", bufs=1, space="PSUM"))
                ps_c = p1.enter_context(
                    tc.tile_pool(name="p1c", bufs=1, space="PSUM"))

                wk_sb = [wp.tile([P, NCH * E], F8, tag=f"wk{i}",
                                 name=f"wk{i}") for i in range(2)]
                wq_sb = [wp.tile([P, NCH * E], F8, tag=f"wq{i}",
                                 name=f"wq{i}") for i in range(2)]
                wv_sb = [wp.tile([P, NCH * E], F8, tag=f"wv{i}",
                                 name=f"wv{i}") for i in range(2)]
                wo_sb = wp.tile([P, NCH * E], F16, tag="wo", name="wo")

                # x tile DMA helper
                def load_x(t):
                    xq = [xp.tile([P, NCH * T1], F8, tag=f"xq{i}",
                                  name=f"xq{i}") for i in range(2)]
                    dxq = [xp.tile([P, E], F8, tag=f"dxq{i}",
                                   name=f"dxq{i}") for i in range(2)]
                    x8 = xp.tile([P, NCH * T1], F16, tag="x8", name="x8")
                    dma(xq[0][:], _ap(xhi8.ap(), t * NCH * P * T1,
                                      [[T1, P], [P * T1, NCH], [1, T1]]))
                    for i, dr in enumerate((dxhi8, dxlo8)):
                        dma(dxq[i][:], _ap(dr.ap(), t * NCH * P * H1,
                                           [[H1, P], [P * H1, NCH],
                                            [1, H1]]))
                    dma(xq[1][:], _ap(xlo8.ap(), t * NCH * P * T1,
                                      [[T1, P], [P * T1, NCH], [1, T1]]))
                    dma(x8[:], _ap(xprep.ap(), t * NCH * P * T1,
                                   [[T1, P], [P * T1, NCH], [1, T1]]))
                    return x8, xq, dxq

                # prefetch queue: (pieces to interleave after each tile)
                PFW = NCH * HID // 8      # 2048-col pieces
                pf = []
                for i in range(2):
                    pf += [(w1_sb0[i], w18[0][i], k) for k in range(8)]
                for i in range(2):
                    pf += [(w2_sb0[i], w28[0][i], k) for k in range(8)]

                def prefetch(n):
                    for _ in range(n):
                        if not pf:
                            return
                        t_sb, t_dr, k = pf.pop(0)
                        dma(t_sb[:, k * PFW:(k + 1) * PFW],
                            _ap(t_dr.ap(), k * PFW,
                                [[NCH * HID, P], [1, PFW]]))

                # initial loads: x0 + wk first so PE can start ASAP;
                # remaining consts slot in after wq.
                x8_t = load_x(0)
                dma(wk_sb[0][:], wk8[0].ap())
                dma(wk_sb[1][:], wk8[1].ap())
                dma(wq_sb[0][:], wq8[0].ap())
                dma(wq_sb[1][:], wq8[1].ap())
                for t_sb, t_dr in ((onesR_sb, onesR), (ones16_sb, ones16),
                                   (hsel_sb, hsel), (hselT_sb, hselT),
                                   (cqsel_sb, cqsel)):
                    dma(t_sb[:], t_dr.ap())
                for i in range(2):
                    dma(wv_sb[i][:], wv8[i].ap())
                dma(cov_sb[:], cov.ap())
                dma(wo_sb[:], wobig.ap())
                for f in range(2):
                    dma(b1_sb[f][:], b1f[f].ap())
                    dma(b2_sb[f][:], b2f[f].ap())

                deferred = [None]  # stats2 emission for previous tile

                def emit_stats2(prev):
                    if prev is None:
                        return
                    out1, sq2, t = prev
                    st2 = ps_st.tile([1, 2 * T1], F32, tag="st", name="st2")
                    for c in range(NCH):
                        MM(st2[:, 0:T1], onesP_sb[:],
                           out1[:, c * T1:(c + 1) * T1],
                           start=(c == 0), stop=False)
                        MM(st2[:, T1:2 * T1], onesP_sb[:],
                           sq2[:, c * T1:(c + 1) * T1],
                           start=False, stop=(c == NCH - 1))
                    stc2 = rwp.tile([1, 2 * T1], F32, tag="stc", name="stc2")
                    nc.scalar.copy(out=stc2[:], in_=st2[:])
                    s12 = rwp.tile([1, T1], F32, tag="s12", name="s12b")
                    nc.vector.tensor_add(
                        s12[:],
                        _tap(stc2, 0, [[T1, 2], [1, H1]]),
                        _tap(stc2, H1, [[T1, 2], [1, H1]]))
                    mmsq = rwp.tile([1, T1], F32, tag="mmsq", name="mmsqb")
                    nc.vector.tensor_scalar_mul(out=mmsq[:], in0=s12[:],
                                                scalar1=1.0 / (S * E))
                    m2t = rwp.tile([1, H1], F32, tag="m2t", name="m2tb")
                    nc.vector.tensor_mul(m2t[:], mmsq[:, 0:H1], mmsq[:, 0:H1])
                    var = rwp.tile([1, H1], F32, tag="var", name="varb")
                    nc.vector.tensor_sub(var[:], mmsq[:, H1:T1], m2t[:])
                    sd = rwp.tile([1, H1], F32, tag="sd", name="sdb")
                    nc.scalar.activation(out=sd[:], in_=var[:], func=AT.Sqrt,
                                         bias=eps_sb[:])
                    r2v = rwp.tile([1, H1], F32, tag="rr", name="r2v")
                    nc.vector.reciprocal(out=r2v[:], in_=sd[:])
                    nc.vector.tensor_copy(out=r2row[:, t * H1:(t + 1) * H1],
                                          in_=r2v[:])
                    r2m2 = rwp.tile([1, H1], F32, tag="rmu", name="r2m2")
                    nc.vector.tensor_mul(r2m2[:], r2v[:], mmsq[:, 0:H1])
                    nc.vector.tensor_scalar_mul(
                        out=nr2m2[:, t * H1:(t + 1) * H1], in0=r2m2[:],
                        scalar1=-1.0)

                for t in range(NT):
                    x8, xq, dxq = x8_t
                    if t + 1 < NT:
                        x8_t = load_x(t + 1)

                    # ---- LN1 stats ----
                    sq8 = sq8p.tile([P, NCH * T1], F8, tag="sq8", name="sq8")
                    nc.scalar.activation(out=sq8[:], in_=xq[0][:],
                                         func=AT.Square)
                    st = ps_st.tile([16, 2 * T1], F32, tag="st", name="st")
                    o8ap = _tap(ones8_sb, 0, [[16, 2], [1, 16]])
                    for cp in range(NCH // 2):
                        MM(_tap(st, 0, [[1, T1]]), o8ap,
                           _tap(xq[0], cp * 2 * T1, [[T1, 2], [1, T1]]),
                           start=(cp == 0), stop=False,
                           perf_mode=mybir.MatmulPerfMode.DoubleRow)
                    for cp in range(NCH // 2):
                        MM(_tap(st, T1, [[1, T1]]), o8ap,
                           start=False, stop=(cp == NCH // 2 - 1),
                           rhs=_tap(sq8, cp * 2 * T1, [[T1, 2], [1, T1]]),
                           perf_mode=mybir.MatmulPerfMode.DoubleRow)
                    stc = rwp.tile([1, 2 * T1], F32, tag="stc", name="stc")
                    stb = st[:]
                    st0 = bass.AP(tensor=stb.tensor, offset=stb.offset,
                                  ap=[[stb.ap[0][0], 1], [1, 2 * T1]])
                    nc.scalar.copy(out=stc[:], in_=st0)

                    # ---- LN1 row math ----
                    s12 = rwp.tile([1, T1], F32, tag="s12", name="s12")
                    nc.vector.tensor_add(
                        s12[:],
                        _tap(stc, 0, [[T1, 2], [1, H1]]),
                        _tap(stc, H1, [[T1, 2], [1, H1]]))
                    mmsq = rwp.tile([1, T1], F32, tag="mmsq", name="mmsq")
                    nc.vector.tensor_scalar_mul(out=mmsq[:], in0=s12[:],
                                                scalar1=1.0 / (S * E))
                    m2t = rwp.tile([1, H1], F32, tag="m2t", name="m2t")
                    nc.vector.tensor_mul(m2t[:], mmsq[:, 0:H1], mmsq[:, 0:H1])
                    var = rwp.tile([1, H1], F32, tag="var", name="var")
                    nc.vector.tensor_sub(var[:], mmsq[:, H1:T1], m2t[:])
                    sd = rwp.tile([1, H1], F32, tag="sd", name="sd")
                    nc.scalar.activation(out=sd[:], in_=var[:], func=AT.Sqrt,
                                         bias=eps_sb[:])
                    rr = rwp.tile([1, H1], F32, tag="rr", name="rr")
                    nc.vector.reciprocal(out=rr[:], in_=sd[:])
                    rf2 = rvp.tile([1, T1], F16, tag="rf2", name="rf2")
                    nc.vector.tensor_copy(out=rf2[:, 0:H1], in_=rr[:])
                    nc.vector.tensor_copy(out=rf2[:, H1:T1], in_=rr[:])
                    r2f = rwp.tile([1, H1], F16, tag="r2f", name="r2f")
                    nc.vector.tensor_mul(r2f[:], rr[:], rr[:])
                    mf16 = rwp.tile([1, H1], F16, tag="mf16", name="mf16")
                    nc.vector.tensor_copy(out=mf16[:], in_=mmsq[:, 0:H1])
                    rmu = rwp.tile([1, H1], F32, tag="rmu", name="rmu")
                    nc.vector.tensor_mul(rmu[:], rr[:], mmsq[:, 0:H1])
                    nrmu2 = rvp.tile([1, T1], F16, tag="nrmu2", name="nrmu2")
                    nc.vector.tensor_scalar_mul(out=nrmu2[:, 0:H1],
                                                in0=rmu[:], scalar1=-1.0)
                    nc.vector.tensor_copy(out=nrmu2[:, H1:T1],
                                          in_=nrmu2[:, 0:H1])

                    # ---- dK = Wk @ dx  (fp8 DoubleRow, 3-set split;
                    # the Act drain descales by 1/(SK*SQ) so the later
                    # q*dK products need no extra scaling) ----
                    DR = mybir.MatmulPerfMode.DoubleRow
                    SETS = ((0, 0), (1, 0), (0, 1))   # (w_idx, x_idx)
                    dkt = dkp.tile([P, E], F16, tag="dkt", name="dkt")
                    for half in range(2):
                        acc = pp.tile([P, 2 * T1], F32, tag="pj", name="kq")
                        nmm = 0
                        for mi in range(4):
                            mcol = (half * 4 + mi) * P
                            for wi, xi in SETS:
                                for kp in range(NCH // 2):
                                    nmm += 1
                                    MM(_tap(acc, mi * H1, [[1, H1]]),
                                       _tap(wk_sb[wi], 2 * kp * E + mcol,
                                            [[E, 2], [1, P]]),
                                       _tap(dxq[xi], 2 * kp * H1,
                                            [[H1, 2], [1, H1]]),
                                       start=(nmm == 1), stop=(nmm == 48),
                                       perf_mode=DR)
                        nc.scalar.activation(
                            out=dkt[:, half * 2 * T1:(half + 1) * 2 * T1],
                            in_=acc[:], func=AT.Copy,
                            scale=1.0 / (SK * SQ))

                    # ---- Q projection + q*dK products ----
                    pr = prp.tile([P, NCH * T1], F16, tag="pr", name="pr")
                    for q in range(4):
                        acc = pp.tile([P, 2 * T1], F32, tag="pj", name="qq")
                        nmm = 0
                        for mi in range(2):
                            mcol = (2 * q + mi) * P
                            for wi, xi in SETS:
                                for kp in range(NCH // 2):
                                    nmm += 1
                                    MM(_tap(acc, mi * T1, [[1, T1]]),
                                       _tap(wq_sb[wi], 2 * kp * E + mcol,
                                            [[E, 2], [1, P]]),
                                       _tap(xq[xi], 2 * kp * T1,
                                            [[T1, 2], [1, T1]]),
                                       start=(nmm == 1), stop=(nmm == 24),
                                       perf_mode=DR)
                        nc.vector.tensor_mul(
                            pr[:, q * 2 * T1:(q + 1) * 2 * T1],
                            acc[:],
                            _tap(dkt, q * T1, [[H1, 2], [0, 2], [1, H1]]))

                    # ---- scores ----
                    at = ps_a.tile([16, 4 * H1], F32, tag="at", name="at")
                    # at[:, 0:T1] = S~d ; at[:, T1:T1+H1] = Ad ;
                    # [T1+H1:..+3*H1] mu16/r216 written by rank-1 MMs below
                    for c in range(NCH):
                        MM(at[:, 0:T1], hsel_sb[:],
                           pr[:, c * T1:(c + 1) * T1],
                           start=(c == 0), stop=False)
                    for c in range(NCH):
                        MM(at[:, T1:T1 + H1],
                           cqsel_sb[:, c * 16:(c + 1) * 16],
                           dkt[:, c * H1:(c + 1) * H1],
                           start=False, stop=False)
                    MM(at[:, T1 + H1:T1 + 2 * H1], ones16_sb[:], mf16[:],
                       start=False, stop=True)
                    ps_r16 = ps_b.tile([16, 2 * H1], F32, tag="r16",
                                       name="r16")
                    MM(ps_r16[:, 0:H1], ones16_sb[:], r2f[:],
                       start=True, stop=False)
                    MM(ps_r16[:, H1:2 * H1], ones16_sb[:], rf2[:, 0:H1],
                       start=False, stop=True)
                    rpl = ps_c.tile([P, 2 * T1], F32, tag="rpl", name="rpl")
                    MM(rpl[:, 0:T1], onesR_sb[:], rf2[:],
                       start=True, stop=True)

                    # D = r^2*(S~d - m*Ad) ; p = sigmoid(D) ; p_r = p*r
                    adls = smp.tile([16, H1], F32, tag="adls",
                                    name="adls")
                    nc.scalar.copy(out=adls[:], in_=at[:, T1:T1 + H1])
                    t1 = smp.tile([16, H1], F32, tag="t1", name="t1")
                    nc.vector.tensor_mul(t1[:], adls[:],
                                         at[:, T1 + H1:T1 + 2 * H1])
                    d1 = smp.tile([16, T1], F32, tag="d1", name="d1")
                    nc.vector.tensor_sub(d1[:], at[:, 0:T1],
                                         _tap(t1, 0, [[0, 2], [1, H1]]))
                    d2 = smp.tile([16, T1], F32, tag="d2", name="d2")
                    nc.vector.tensor_mul(d2[:], d1[:],
                                         _tap(ps_r16, 0, [[0, 2], [1, H1]]))
                    psig = smp.tile([16, T1], F16, tag="psig", name="psig")
                    nc.scalar.activation(out=psig[:], in_=d2[:],
                                         func=AT.Sigmoid)
                    p_r = smp.tile([16, T1], F16, tag="p_r", name="p_r")
                    nc.vector.tensor_mul(p_r[:], psig[:],
                                         _tap(ps_r16, H1, [[0, 2], [1, H1]]))

                    # ---- V projection + dV / V1r + Z pieces ----
                    # px (p_r expanded over features) lives in rpl's second
                    # column half (one PSUM bank for both); it is emitted
                    # after the first V quarter so Z pieces overlap the
                    # remaining V matmuls.
                    dvt = dkp.tile([P, E], F16, tag="dvt", name="dvt")
                    v1r = dkp.tile([P, E], F16, tag="v1r", name="v1r")
                    z = zp.tile([P, NCH * T1], F16, tag="z", name="z")
                    for q in range(4):
                        acc = pp.tile([P, 2 * T1], F32, tag="pj", name="vq")
                        nmm = 0
                        for mi in range(2):
                            mcol = (2 * q + mi) * P
                            for wi, xi in SETS:
                                for kp in range(NCH // 2):
                                    nmm += 1
                                    MM(_tap(acc, mi * T1, [[1, T1]]),
                                       _tap(wv_sb[wi], 2 * kp * E + mcol,
                                            [[E, 2], [1, P]]),
                                       _tap(xq[xi], 2 * kp * T1,
                                            [[T1, 2], [1, T1]]),
                                       start=(nmm == 1), stop=(nmm == 24),
                                       perf_mode=DR)
                        if q == 0:
                            MM(rpl[:, T1:2 * T1], hselT_sb[:], p_r[:],
                               start=True, stop=True)
                        vs = dkp.tile([P, 2 * T1], F16, tag="vs", name="vs")
                        nc.scalar.activation(out=vs[:], in_=acc[:],
                                             func=AT.Copy, scale=1.0 / SV)
                        nc.vector.tensor_sub(
                            _tap(dvt, q * T1, [[H1, 2], [1, H1]]),
                            _tap(vs, 0, [[T1, 2], [1, H1]]),
                            _tap(vs, H1, [[T1, 2], [1, H1]]))
                        nc.vector.tensor_mul(
                            _tap(v1r, q * T1, [[H1, 2], [1, H1]]),
                            _tap(vs, H1, [[T1, 2], [1, H1]]),
                            _tap(rpl, 0, [[0, 2], [1, H1]]))
                        for s in range(2):
                            zqt = ztp.tile([P, T1], F16, tag="zqt",
                                           name="zqt")
                            nc.vector.scalar_tensor_tensor(
                                out=zqt[:],
                                in0=_tap(dvt, q * T1, [[H1, 2], [1, H1]]),
                                scalar=1.0,
                                in1=_tap(rpl, T1 + s * H1,
                                         [[0, 2], [1, H1]]),
                                op0=OP.mult, op1=OP.mult)
                            nc.vector.tensor_add(
                                _tap(z, q * 2 * T1 + s * H1,
                                     [[T1, 2], [1, H1]]),
                                zqt[:],
                                _tap(v1r, q * T1, [[H1, 2], [1, H1]]))

                    # ---- O-proj (+ rank-1 -r*m*co correction) ----
                    oacc = [pp.tile([P, 2 * T1], F32, tag="pj", name="oq")
                            for q in range(4)]
                    for c in range(NCH):
                        for q in range(4):
                            for mi in range(2):
                                m = 2 * q + mi
                                MM(oacc[q][:, mi * T1:(mi + 1) * T1],
                                   wo_sb[:, c * E + m * P:c * E + m * P + P],
                                   z[:, c * T1:(c + 1) * T1],
                                   start=(c == 0 and mi == 0), stop=False)
                    for q in range(4):
                        for mi in range(2):
                            m = 2 * q + mi
                            MM(oacc[q][:, mi * T1:(mi + 1) * T1],
                               cov_sb[:, m * P:(m + 1) * P], nrmu2[:],
                               start=False, stop=(mi == 1))

                    # ---- residual add -> out1 (fp16) ; spill ----
                    out1 = o1p.tile([P, NCH * T1], F16, tag="o1", name="o1")
                    for q in range(4):
                        nc.vector.tensor_add(
                            out1[:, q * 2 * T1:(q + 1) * 2 * T1],
                            oacc[q][:],
                            x8[:, q * 2 * T1:(q + 1) * 2 * T1])
                    for f2 in range(2):
                        dma(_ap(o1T.ap(), f2 * NCH * P * ROWS + t * H1,
                                [[ROWS, P], [P * ROWS, NCH], [1, H1]]),
                            _tap(out1, f2 * H1, [[T1, NCH], [1, H1]]))

                    # ---- LN2: square now, stats deferred ----
                    sq2 = sqp.tile([P, NCH * T1], F16, tag="sq2", name="sq2")
                    nc.scalar.activation(out=sq2[:], in_=out1[:],
                                         func=AT.Square)
                    emit_stats2(deferred[0])
                    deferred[0] = (out1, sq2, t)

                    if DEBUG and t == 0:
                        for nm, tl in (("stc", stc), ("rf2", rf2),
                                       ("dkt", dkt), ("d2", d2),
                                       ("p_r", p_r), ("z", z),
                                       ("nrmu2", nrmu2), ("v1r", v1r),
                                       ("rmu", rmu), ("mmsq", mmsq),
                                       ("rr", rr)):
                            dma(dbg[nm].ap(), tl[:])

                    prefetch(4)

                emit_stats2(deferred[0])
                prefetch(16)

            # =================== PASS 2 ===================
            with ExitStack() as p2:
                wff1 = p2.enter_context(tc.tile_pool(name="wff1", bufs=1))
                o1lp = p2.enter_context(tc.tile_pool(name="p2o1", bufs=3))
                osp = p2.enter_context(tc.tile_pool(name="p2os", bufs=2))
                hp = p2.enter_context(tc.tile_pool(name="p2h", bufs=2))
                yp = p2.enter_context(tc.tile_pool(name="p2y", bufs=2))
                fp = p2.enter_context(tc.tile_pool(name="p2f", bufs=2))
                ps_h = p2.enter_context(
                    tc.tile_pool(name="p2psh", bufs=4, space="PSUM"))
                ps_y = p2.enter_context(
                    tc.tile_pool(name="p2psy", bufs=2, space="PSUM"))
                ps_r = p2.enter_context(
                    tc.tile_pool(name="p2psr", bufs=2, space="PSUM"))

                w1_sb1 = [wff1.tile([P, NCH * HID], F8, tag=f"w1{i}",
                                    name=f"w1s1{i}") for i in range(2)]
                w2_sb1 = [wff1.tile([P, NHID * E], F8, tag=f"w2{i}",
                                    name=f"w2s1{i}") for i in range(2)]
                PFW = NCH * HID // 8
                pf2 = []
                for i in range(2):
                    pf2 += [(w1_sb1[i], w18[1][i], k) for k in range(8)]
                for i in range(2):
                    pf2 += [(w2_sb1[i], w28[1][i], k) for k in range(8)]

                def prefetch2(n):
                    for _ in range(n):
                        if not pf2:
                            return
                        t_sb, t_dr, k = pf2.pop(0)
                        dma(t_sb[:, k * PFW:(k + 1) * PFW],
                            _ap(t_dr.ap(), k * PFW,
                                [[NCH * HID, P], [1, PFW]]))

                tiles = [(f, u) for f in range(2) for u in range(NU)]

                def stage(i):
                    """Issue o1 load + r2 plane + r2-scale for tile i."""
                    f, u = tiles[i]
                    o1f = o1lp.tile([P, NCH * T2], F16, tag="o1f",
                                    name="o1f")
                    for ch in range(2):
                        nc.scalar.dma_start(
                            out=o1f[:, ch * 4 * T2:(ch + 1) * 4 * T2],
                            in_=_ap(o1T.ap(),
                                    (f * NCH + ch * 4) * P * ROWS + u * T2,
                                    [[ROWS, P], [P * ROWS, 4], [1, T2]]))
                    prefetch2(8)
                    r2pl = ps_r.tile([P, 2 * T2], F32, tag="r2pl",
                                     name="r2pl")
                    MM(r2pl[:, 0:T2], onesR_sb[:],
                       r2row[:, u * T2:(u + 1) * T2],
                       start=True, stop=False)
                    MM(r2pl[:, T2:2 * T2], onesR_sb[:],
                       nr2m2[:, u * T2:(u + 1) * T2],
                       start=False, stop=True)
                    osr = osp.tile([P, NCH * T2], F8, tag="osr",
                                   name="osr")
                    for q2 in range(4):
                        ost = osp.tile([P, 2 * T2], F16, tag="ost",
                                       name="ost")
                        nc.vector.scalar_tensor_tensor(
                            out=ost[:],
                            in0=_tap(o1f, q2 * 2 * T2, [[1, 2 * T2]]),
                            scalar=1.0,
                            in1=_tap(r2pl, 0, [[0, 2], [1, T2]]),
                            op0=OP.mult, op1=OP.mult)
                        nc.vector.tensor_add(
                            _tap(osr, q2 * 2 * T2, [[1, 2 * T2]]),
                            ost[:],
                            _tap(r2pl, T2, [[0, 2], [1, T2]]))
                    return o1f, osr

                staged = stage(0)
                for i in range(len(tiles)):
                    f, u = tiles[i]
                    o1f, osr = staged
                    if i + 1 < len(tiles):
                        staged = stage(i + 1)
                    w1s = w1_sb0 if f == 0 else w1_sb1
                    w2s = w2_sb0 if f == 0 else w2_sb1
                    DR = mybir.MatmulPerfMode.DoubleRow

                    h = hp.tile([P, NHID * T2], F8, tag="h", name="h")
                    for kc in range(NHID):
                        hacc = ps_h.tile([P, T2], F32, tag="hacc",
                                         name="hacc")
                        nmm = 0
                        for wi in range(2):
                            for cp in range(NCH // 2):
                                nmm += 1
                                MM(hacc[:],
                                   _tap(w1s[wi], 2 * cp * HID + kc * P,
                                        [[HID, 2], [1, P]]),
                                   _tap(osr, 2 * cp * T2,
                                        [[T2, 2], [1, T2]]),
                                   start=(nmm == 1), stop=(nmm == 8),
                                   perf_mode=DR)
                        nc.scalar.activation(
                            out=h[:, kc * T2:(kc + 1) * T2], in_=hacc[:],
                            func=AT.Tanh, bias=b1_sb[f][:, kc:kc + 1],
                            scale=1.0 / SW1)

                    y = yp.tile([P, NCH * T2], F16, tag="y", name="y")
                    for mp_ in range(4):
                        ys = ps_y.tile([P, 2 * T2], F32, tag="ys",
                                       name="ys")
                        for mb in range(2):
                            m = 2 * mp_ + mb
                            nmm = 0
                            for wi in range(2):
                                for kcp in range(NHID // 2):
                                    nmm += 1
                                    MM(_tap(ys, mb * T2, [[1, T2]]),
                                       _tap(w2s[wi], kcp * 2 * E + m * P,
                                            [[E, 2], [1, P]]),
                                       _tap(h, kcp * 2 * T2,
                                            [[T2, 2], [1, T2]]),
                                       start=(nmm == 1), stop=(nmm == 16),
                                       perf_mode=DR)
                        for mb in range(2):
                            m = 2 * mp_ + mb
                            nc.scalar.activation(
                                out=y[:, m * T2:(m + 1) * T2],
                                in_=ys[:, mb * T2:(mb + 1) * T2],
                                func=AT.Tanh, bias=b2_sb[f][:, m:m + 1],
                                scale=1.0 / SW2)

                    fin = fp.tile([P, NCH * T2], F32, tag="fin",
                                  name="fin")
                    for mp_ in range(4):
                        sl = slice(mp_ * 2 * T2, (mp_ + 1) * 2 * T2)
                        nc.vector.tensor_add(fin[:, sl], y[:, sl],
                                             o1f[:, sl])
                        dma(_ap(outT.ap(),
                                (f * NU + u) * NCH * P * T2
                                + mp_ * 2 * P * T2,
                                [[T2, P], [P * T2, 2], [1, T2]]),
                            fin[:, sl])
    return nc


_NC_CACHE = None


def _get_nc():
    global _NC_CACHE
    if _NC_CACHE is None:
        nc = bacc.Bacc("TRN2", target_bir_lowering=False, debug=False)
        build(nc)
        nc.compile()
        _NC_CACHE = nc
    return _NC_CACHE


def _prep_shared(inputs):
    f32, f16 = np.float32, np.float16
    f8 = ml_dtypes.float8_e4m3fn
    d = {}
    assert np.all(np.asarray(inputs["ln1_b"]) == 0), "ln1_b must be zero"
    assert np.all(np.asarray(inputs["ln2_b"]) == 0), "ln2_b must be zero"
    ln1w = np.asarray(inputs["ln1_w"], f32)
    ln2w = np.asarray(inputs["ln2_w"], f32)
    assert np.array_equal(ln1w[0], ln1w[1]), "ln1_w must match across s"

    def split8(W, s):
        """Scaled fp8 hi/lo split; returns (hi8, lo8, effective_W)."""
        Ws = W * s
        hi = Ws.astype(f8)
        lo = (Ws - hi.astype(f32)).astype(f8)
        eff = (hi.astype(f32) + lo.astype(f32)) / s
        return hi, lo, eff

    def wbig(w):
        # lhsT layout: arr[p, c*E + m*128 + mm] = w[m*128+mm, c*128+p]
        Eo, Ei = w.shape
        return np.ascontiguousarray(
            w.reshape(Eo // P, P, Ei // P, P).transpose(3, 2, 0, 1)
            .reshape(P, Ei // P * Eo))

    Wq = np.asarray(inputs["Wq"], f32) * (ln1w[0] / 8.0)[None, :]
    Wk = np.asarray(inputs["Wk"], f32) * ln1w[0][None, :]
    Wv = np.asarray(inputs["Wv"], f32) * ln1w[0][None, :]
    Wo = np.asarray(inputs["Wo"], f32)

    qhi, qlo, qeff = split8(Wq, SQ)
    khi, klo, _ = split8(Wk, SK)
    vhi, vlo, veff = split8(Wv, SV)
    for i, a in enumerate((qhi, qlo)):
        d[f"wq8{i}"] = wbig(a)
    for i, a in enumerate((khi, klo)):
        d[f"wk8{i}"] = wbig(a)
    for i, a in enumerate((vhi, vlo)):
        d[f"wv8{i}"] = wbig(a)
    Woh = Wo.astype(f16).astype(f32)
    d["wobig"] = wbig(Wo).astype(f16)

    cqv = qeff.sum(1)                   # [E] effective row sums
    cvv = veff.sum(1)
    cov = Woh @ cvv
    d["cov"] = cov.reshape(1, E).astype(f16)

    cqsel = np.zeros((P, NCH * 16), f32)
    for c in range(NCH):
        dd = np.arange(P)
        cqsel[dd, c * 16 + (dd % 16)] = cqv[c * P + dd] * SQ
    d["cqsel"] = cqsel.astype(f16)
    hs = np.zeros((P, 16), f32)
    hs[np.arange(P), np.arange(P) % 16] = 1.0
    d["hsel"] = hs.astype(f16)
    d["hselT"] = np.ascontiguousarray(hs.T).astype(f16)
    d["onesP"] = np.ones((P, 1), f16)
    d["ones8"] = np.ones((P, 32), ml_dtypes.float8_e4m3fn)
    d["onesR"] = np.ones((1, P), f16)
    d["ones16"] = np.ones((1, 16), f16)

    for f in range(2):
        W1 = np.asarray(inputs["f1w1" if f == 0 else "f2w1"], f32) \
            * ln2w[f][None, :]
        W2 = np.asarray(inputs["f1w2" if f == 0 else "f2w2"], f32)
        B1 = np.asarray(inputs["f1b1" if f == 0 else "f2b1"], f32)
        B2 = np.asarray(inputs["f1b2" if f == 0 else "f2b2"], f32)
        w1hi, w1lo, _ = split8(W1, SW1)
        w2hi, w2lo, _ = split8(W2, SW2)
        # w1big: arr[p, c*HID + kc*128 + mm] = W1[kc*128+mm, c*128+p]
        for i, a in enumerate((w1hi, w1lo)):
            d[f"w18{f}{i}"] = np.ascontiguousarray(
                a.reshape(NHID, P, NCH, P).transpose(3, 2, 0, 1)
                .reshape(P, NCH * HID))
        # w2big: arr[p, kc*E + m*128 + mm] = W2[m*128+mm, kc*128+p]
        for i, a in enumerate((w2hi, w2lo)):
            d[f"w28{f}{i}"] = np.ascontiguousarray(
                a.reshape(NCH, P, NHID, P).transpose(3, 2, 0, 1)
                .reshape(P, NHID * E))
        d[f"b1f{f}"] = np.ascontiguousarray(
            B1.reshape(NHID, P).T).astype(f32)
        d[f"b2f{f}"] = np.ascontiguousarray(
            B2.reshape(NCH, P).T).astype(f32)
    return d


def _prep_core(inputs, core):
    f32 = np.float32
    f8 = ml_dtypes.float8_e4m3fn
    rows0 = core * ROWS
    chunk = np.asarray(inputs["input"][rows0:rows0 + ROWS], f32)
    # [t, j, s, c, p] -> [t, c, p, s, j]
    arr = chunk.reshape(NT, H1, S, NCH, P).transpose(0, 3, 4, 2, 1)
    xprep = np.ascontiguousarray(arr.reshape(NT * NCH * P, T1)) \
        .astype(np.float16)
    xf = xprep.astype(f32)
    xhi = xf.astype(f8)
    xlo = (xf - xhi.astype(f32)).astype(f8)
    # dx[t, c, p, j] = x[.., s=0] - x[.., s=1]
    dx = (arr[:, :, :, 0, :] - arr[:, :, :, 1, :]).reshape(NT * NCH * P, H1)
    dxhi = dx.astype(f8)
    dxlo = (dx - dxhi.astype(f32)).astype(f8)
    return {"xprep": xprep, "xhi8": xhi, "xlo8": xlo,
            "dxhi8": dxhi, "dxlo8": dxlo}


def _decode_out(outT):
    # outT rows: (((f*NU + u)*NCH + c)*P + p), cols j
    arr = outT.reshape(2, NU, NCH, P, T2)
    out = np.empty((ROWS, S, E), np.float32)
    for f in range(2):
        for u in range(NU):
            # arr[f,u,c,p,j] -> out[u*T2+j, f, c*128+p]
            out[u * T2:(u + 1) * T2, f, :] = \
                arr[f, u].transpose(2, 0, 1).reshape(T2, E)
    return out


def kernel(**inputs):
    nc = _get_nc()
    shared = _prep_shared(inputs)
    in_maps = []
    for core in range(NCORES):
        m = dict(shared)
        m.update(_prep_core(inputs, core))
        in_maps.append(m)
    res = bass_utils.run_bass_kernel_spmd(nc, in_maps,
                                          core_ids=list(range(NCORES)))
    out = np.empty((B, S, E), np.float32)
    for core in range(NCORES):
        out[core * ROWS:(core + 1) * ROWS] = _decode_out(
            res.results[core]["outT"])
    return out



# revision 5
# speedup vs baseline: 1.0674x; 1.0674x over previous
"""Trainium2 Bass kernel for nn_InterFusion (dense transformer block, S=2).

Strategy (v2):
  - Pure data parallel: batch (8192) split across 8 NeuronCores; weights
    replicated, fp16 on-chip (PSUM accumulation stays fp32).
  - Feature-major layout: activations live as X^T tiles [feat(128) x tokens].
  - Both LayerNorms are folded algebraically into the matmuls so the heavy
    projections run on RAW (un-normalized) activations with no dependency on
    the LN statistics:
      LN1:  q = r*(Wq@x) - r*m*rowsum(Wq)  (same for k,v).  The softmax needs
            only score DIFFERENCES over t, so per-head corrections reduce to
            D[h,s] = r^2*(S~d[h,s] - m*Ad[h]) with S~d from q*(K0-K1) products
            and Ad from a cq-masked head-sum of (K0-K1); 2-way softmax is a
            single Sigmoid.  V-side corrections collapse into a rank-1 (K=1)
            matmul accumulated straight into the O-projection PSUM.
      LN2:  pass 2 scales o1 by r2 (per-column) before the FFN W1 matmul and
            accumulates the -r2*m2*rowsum(W1) correction as a K=1 matmul.
  - Two passes over DRAM: pass1 = LN1+QKV+attn+O-proj+residual (+LN2 stats),
    spilling out1 (fp16); pass2 = per-position FFN + final residual.
    FFN weights for position 0 prefetch during pass 1, position 1 during
    pass 2a, so the PE never waits on weight DMA.
"""

import sys

for _p in ("/opt/trn_rl_repo", "/root/.axon_site/_ro/trn_rl_repo"):
    if _p not in sys.path:
        sys.path.append(_p)

import ml_dtypes
import numpy as np

import concourse.bass as bass
import concourse.bacc as bacc
import concourse.tile as tile
from concourse import mybir
from concourse import bass_utils

F32 = mybir.dt.float32
F16 = mybir.dt.float16
F8 = mybir.dt.float8e4
AT = mybir.ActivationFunctionType
OP = mybir.AluOpType

DEBUG = False

E = 1024
S = 2
B = 8192
NCORES = 8
ROWS = B // NCORES          # 1024 rows per core
P = 128
NCH = E // P                # 8 feature chunks
HID = 2 * E
NHID = HID // P             # 16 hidden chunks
EPS = 1e-5

SQ = 128.0                  # fp8 pre-scale for Wq (incl /8 fold)
SK = 32.0                   # fp8 pre-scale for Wk
SV = 32.0                   # fp8 pre-scale for Wv
SW1 = 32.0                  # fp8 pre-scale for W1
SW2 = 32.0                  # fp8 pre-scale for W2

H1 = 128                    # pass-1: rows per tile
T1 = 2 * H1                 # 256 cols (s-major: s*128 + j)
NT = ROWS // H1             # 8 tiles
T2 = 256                    # pass-2: rows per tile (single position)
NU = ROWS // T2             # 4 tiles per position

QKV_SETS = 3                # fp8 (w,x) product terms for Q/K/V projections
W1_SETS = 1                 # fp8 weight sets for FFN layer 1
W2_SETS = 1                 # fp8 weight sets for FFN layer 2


def _ap(handle_ap, offset, dims):
    return bass.AP(tensor=handle_ap.tensor, offset=handle_ap.offset + offset,
                   ap=[list(d) for d in dims])


def _tap(t, offset, dims):
    """AP on SBUF/PSUM tile t with explicit free dims; partition dim kept."""
    base = t[:]
    pdim = base.ap[0]
    return bass.AP(tensor=base.tensor, offset=base.offset + offset,
                   ap=[list(pdim)] + [list(d) for d in dims])


def build(nc):
    # ---------------- DRAM I/O ----------------
    xprep = nc.dram_tensor("xprep", [NT * NCH * P, T1], F16,
                           kind="ExternalInput")
    xhi8 = nc.dram_tensor("xhi8", [NT * NCH * P, T1], F8,
                          kind="ExternalInput")
    xlo8 = nc.dram_tensor("xlo8", [NT * NCH * P, T1], F8,
                          kind="ExternalInput")
    dxhi8 = nc.dram_tensor("dxhi8", [NT * NCH * P, H1], F8,
                           kind="ExternalInput")
    dxlo8 = nc.dram_tensor("dxlo8", [NT * NCH * P, H1], F8,
                           kind="ExternalInput")
    wk8 = [nc.dram_tensor(f"wk8{i}", [P, NCH * E], F8, kind="ExternalInput")
           for i in range(2)]
    wq8 = [nc.dram_tensor(f"wq8{i}", [P, NCH * E], F8, kind="ExternalInput")
           for i in range(2)]
    wv8 = [nc.dram_tensor(f"wv8{i}", [P, NCH * E], F8, kind="ExternalInput")
           for i in range(2)]
    wobig = nc.dram_tensor("wobig", [P, NCH * E], F16, kind="ExternalInput")
    w18 = [[nc.dram_tensor(f"w18{f}{i}", [P, NCH * HID], F8,
                           kind="ExternalInput") for i in range(2)]
           for f in range(2)]
    w28 = [[nc.dram_tensor(f"w28{f}{i}", [P, NHID * E], F8,
                           kind="ExternalInput") for i in range(2)]
           for f in range(2)]
    b1f = [nc.dram_tensor(f"b1f{f}", [P, NHID], F32, kind="ExternalInput")
           for f in range(2)]
    b2f = [nc.dram_tensor(f"b2f{f}", [P, NCH], F32, kind="ExternalInput")
           for f in range(2)]
    hsel = nc.dram_tensor("hsel", [P, 16], F16, kind="ExternalInput")
    hselT = nc.dram_tensor("hselT", [16, P], F16, kind="ExternalInput")
    cqsel = nc.dram_tensor("cqsel", [P, NCH * 16], F16, kind="ExternalInput")
    cov = nc.dram_tensor("cov", [1, E], F16, kind="ExternalInput")
    onesP = nc.dram_tensor("onesP", [P, 1], F16, kind="ExternalInput")
    onesR = nc.dram_tensor("onesR", [1, P], F16, kind="ExternalInput")
    ones16 = nc.dram_tensor("ones16", [1, 16], F16, kind="ExternalInput")
    ones8 = nc.dram_tensor("ones8", [P, 32], F8, kind="ExternalInput")

    o1T = nc.dram_tensor("o1T", [2 * NCH * P, ROWS], F16, kind="Internal")
    if DEBUG:
        dbg = {nm: nc.dram_tensor(f"dbg_{nm}", shp, dt, kind="ExternalOutput")
               for nm, shp, dt in [
                   ("stc", [1, 2 * T1], F32), ("rf2", [1, T1], F16),
                   ("dkt", [P, E], F16), ("d2", [16, T1], F32),
                   ("p_r", [16, T1], F16), ("z", [P, NCH * T1], F16),
                   ("nrmu2", [1, T1], F16), ("v1r", [P, E], F16),
                   ("rmu", [1, H1], F32), ("mmsq", [1, T1], F32),
                   ("rr", [1, H1], F32)]}
    outT = nc.dram_tensor("outT", [2 * NU * NCH * P, T2], F32,
                          kind="ExternalOutput")

    MM = nc.tensor.matmul

    def dma(out, in_):
        nc.sync.dma_start(out=out, in_=in_)

    with tile.TileContext(nc) as tc:
        from contextlib import ExitStack

        with ExitStack() as outer:
            cpool = outer.enter_context(tc.tile_pool(name="c", bufs=1))
            wff0 = outer.enter_context(tc.tile_pool(name="wff0", bufs=1))

            # constants
            hsel_sb = cpool.tile([P, 16], F16, tag="hsel", name="hsel")
            hselT_sb = cpool.tile([16, P], F16, tag="hselT", name="hselT")
            cqsel_sb = cpool.tile([P, NCH * 16], F16, tag="cqsel",
                                  name="cqsel")
            cov_sb = cpool.tile([1, E], F16, tag="cov", name="cov")
            onesP_sb = cpool.tile([P, 1], F16, tag="onesP", name="onesP")
            onesR_sb = cpool.tile([1, P], F16, tag="onesR", name="onesR")
            ones16_sb = cpool.tile([1, 16], F16, tag="ones16", name="ones16")
            ones8_sb = cpool.tile([P, 32], F8, tag="ones8", name="ones8")
            b1_sb = [cpool.tile([P, NHID], F32, tag=f"b1{f}", name=f"b1{f}")
                     for f in range(2)]
            b2_sb = [cpool.tile([P, NCH], F32, tag=f"b2{f}", name=f"b2{f}")
                     for f in range(2)]
            eps_sb = cpool.tile([1, 1], F32, tag="eps", name="eps")
            nc.vector.memset(eps_sb[:], EPS)
            # persistent LN2 rows (written pass 1, read pass 2)
            r2row = cpool.tile([1, ROWS], F16, tag="r2row", name="r2row")
            nr2m2 = cpool.tile([1, ROWS], F16, tag="nr2m2", name="nr2m2")

            dma(onesP_sb[:], onesP.ap())
            dma(ones8_sb[:], ones8.ap())

            # FFN position-0 weights (prefetched during pass 1)
            w1_sb0 = [wff0.tile([P, NCH * HID], F8, tag=f"w1{i}",
                                name=f"w1s0{i}") for i in range(2)]
            w2_sb0 = [wff0.tile([P, NHID * E], F8, tag=f"w2{i}",
                                name=f"w2s0{i}") for i in range(2)]

            # =================== PASS 1 ===================
            with ExitStack() as p1:
                wp = p1.enter_context(tc.tile_pool(name="p1w", bufs=1))
                xp = p1.enter_context(tc.tile_pool(name="p1x", bufs=2))
                sqp = p1.enter_context(tc.tile_pool(name="p1sq", bufs=2))
                sq8p = p1.enter_context(tc.tile_pool(name="p1sq8", bufs=1))
                dkp = p1.enter_context(tc.tile_pool(name="p1dk", bufs=1))
                prp = p1.enter_context(tc.tile_pool(name="p1pr", bufs=1))
                zp = p1.enter_context(tc.tile_pool(name="p1z", bufs=1))
                ztp = p1.enter_context(tc.tile_pool(name="p1zt", bufs=2))
                o1p = p1.enter_context(tc.tile_pool(name="p1o1", bufs=2))
                rwp = p1.enter_context(tc.tile_pool(name="p1row", bufs=1))
                rvp = p1.enter_context(tc.tile_pool(name="p1rv", bufs=2))
                smp = p1.enter_context(tc.tile_pool(name="p1sm", bufs=1))
                pp = p1.enter_context(
                    tc.tile_pool(name="p1mm", bufs=4, space="PSUM"))
                ps_st = p1.enter_context(
                    tc.tile_pool(name="p1st", bufs=1, space="PSUM"))
                ps_a = p1.enter_context(
                    tc.tile_pool(name="p1a", bufs=1, space="PSUM"))
                ps_b = p1.enter_context(
                    tc.tile_pool(name="p1b", bufs=1, space="PSUM"))
                ps_c = p1.enter_context(
                    tc.tile_pool(name="p1c", bufs=1, space="PSUM"))

                wk_sb = [wp.tile([P, NCH * E], F8, tag=f"wk{i}",
                                 name=f"wk{i}") for i in range(2)]
                wq_sb = [wp.tile([P, NCH * E], F8, tag=f"wq{i}",
                                 name=f"wq{i}") for i in range(2)]
                wv_sb = [wp.tile([P, NCH * E], F8, tag=f"wv{i}",
                                 name=f"wv{i}") for i in range(2)]
                wo_sb = wp.tile([P, NCH * E], F16, tag="wo", name="wo")

                # x tile DMA helper
                def load_x(t):
                    xq = [xp.tile([P, NCH * T1], F8, tag=f"xq{i}",
                                  name=f"xq{i}") for i in range(2)]
                    dxq = [xp.tile([P, E], F8, tag=f"dxq{i}",
                                   name=f"dxq{i}") for i in range(2)]
                    x8 = xp.tile([P, NCH * T1], F16, tag="x8", name="x8")
                    dma(xq[0][:], _ap(xhi8.ap(), t * NCH * P * T1,
                                      [[T1, P], [P * T1, NCH], [1, T1]]))
                    for i, dr in enumerate((dxhi8, dxlo8)):
                        dma(dxq[i][:], _ap(dr.ap(), t * NCH * P * H1,
                                           [[H1, P], [P * H1, NCH],
                                            [1, H1]]))
                    dma(xq[1][:], _ap(xlo8.ap(), t * NCH * P * T1,
                                      [[T1, P], [P * T1, NCH], [1, T1]]))
                    dma(x8[:], _ap(xprep.ap(), t * NCH * P * T1,
                                   [[T1, P], [P * T1, NCH], [1, T1]]))
                    return x8, xq, dxq

                # prefetch queue: (pieces to interleave after each tile)
                PFW = NCH * HID // 8      # 2048-col pieces
                pf = []
                for i in range(2):
                    pf += [(w1_sb0[i], w18[0][i], k) for k in range(8)]
                for i in range(2):
                    pf += [(w2_sb0[i], w28[0][i], k) for k in range(8)]

                def prefetch(n):
                    for _ in range(n):
                        if not pf:
                            return
                        t_sb, t_dr, k = pf.pop(0)
                        dma(t_sb[:, k * PFW:(k + 1) * PFW],
                            _ap(t_dr.ap(), k * PFW,
                                [[NCH * HID, P], [1, PFW]]))

                # initial loads: x0 + wk first so PE can start ASAP;
                # remaining consts slot in after wq.
                x8_t = load_x(0)
                dma(wk_sb[0][:], wk8[0].ap())
                dma(wk_sb[1][:], wk8[1].ap())
                dma(wq_sb[0][:], wq8[0].ap())
                dma(wq_sb[1][:], wq8[1].ap())
                for t_sb, t_dr in ((onesR_sb, onesR), (ones16_sb, ones16),
                                   (hsel_sb, hsel), (hselT_sb, hselT),
                                   (cqsel_sb, cqsel)):
                    dma(t_sb[:], t_dr.ap())
                for i in range(2):
                    dma(wv_sb[i][:], wv8[i].ap())
                dma(cov_sb[:], cov.ap())
                dma(wo_sb[:], wobig.ap())
                for f in range(2):
                    dma(b1_sb[f][:], b1f[f].ap())
                    dma(b2_sb[f][:], b2f[f].ap())

                deferred = [None]  # stats2 emission for previous tile

                def emit_stats2(prev):
                    if prev is None:
                        return
                    out1, sq2, t = prev
                    st2 = ps_st.tile([1, 2 * T1], F32, tag="st", name="st2")
                    for c in range(NCH):
                        MM(st2[:, 0:T1], onesP_sb[:],
                           out1[:, c * T1:(c + 1) * T1],
                           start=(c == 0), stop=False)
                        MM(st2[:, T1:2 * T1], onesP_sb[:],
                           sq2[:, c * T1:(c + 1) * T1],
                           start=False, stop=(c == NCH - 1))
                    stc2 = rwp.tile([1, 2 * T1], F32, tag="stc", name="stc2")
                    nc.scalar.copy(out=stc2[:], in_=st2[:])
                    s12 = rwp.tile([1, T1], F32, tag="s12", name="s12b")
                    nc.vector.tensor_add(
                        s12[:],
                        _tap(stc2, 0, [[T1, 2], [1, H1]]),
                        _tap(stc2, H1, [[T1, 2], [1, H1]]))
                    mmsq = rwp.tile([1, T1], F32, tag="mmsq", name="mmsqb")
                    nc.vector.tensor_scalar_mul(out=mmsq[:], in0=s12[:],
                                                scalar1=1.0 / (S * E))
                    m2t = rwp.tile([1, H1], F32, tag="m2t", name="m2tb")
                    nc.vector.tensor_mul(m2t[:], mmsq[:, 0:H1], mmsq[:, 0:H1])
                    var = rwp.tile([1, H1], F32, tag="var", name="varb")
                    nc.vector.tensor_sub(var[:], mmsq[:, H1:T1], m2t[:])
                    sd = rwp.tile([1, H1], F32, tag="sd", name="sdb")
                    nc.scalar.activation(out=sd[:], in_=var[:], func=AT.Sqrt,
                                         bias=eps_sb[:])
                    r2v = rwp.tile([1, H1], F32, tag="rr", name="r2v")
                    nc.vector.reciprocal(out=r2v[:], in_=sd[:])
                    nc.vector.tensor_copy(out=r2row[:, t * H1:(t + 1) * H1],
                                          in_=r2v[:])
                    r2m2 = rwp.tile([1, H1], F32, tag="rmu", name="r2m2")
                    nc.vector.tensor_mul(r2m2[:], r2v[:], mmsq[:, 0:H1])
                    nc.vector.tensor_scalar_mul(
                        out=nr2m2[:, t * H1:(t + 1) * H1], in0=r2m2[:],
                        scalar1=-1.0)

                for t in range(NT):
                    x8, xq, dxq = x8_t
                    if t + 1 < NT:
                        x8_t = load_x(t + 1)

                    # ---- LN1 stats ----
                    sq8 = sq8p.tile([P, NCH * T1], F8, tag="sq8", name="sq8")
                    nc.scalar.activation(out=sq8[:], in_=xq[0][:],
                                         func=AT.Square)
                    st = ps_st.tile([16, 2 * T1], F32, tag="st", name="st")
                    o8ap = _tap(ones8_sb, 0, [[16, 2], [1, 16]])
                    for cp in range(NCH // 2):
                        MM(_tap(st, 0, [[1, T1]]), o8ap,
                           _tap(xq[0], cp * 2 * T1, [[T1, 2], [1, T1]]),
                           start=(cp == 0), stop=False,
                           perf_mode=mybir.MatmulPerfMode.DoubleRow)
                    for cp in range(NCH // 2):
                        MM(_tap(st, T1, [[1, T1]]), o8ap,
                           start=False, stop=(cp == NCH // 2 - 1),
                           rhs=_tap(sq8, cp * 2 * T1, [[T1, 2], [1, T1]]),
                           perf_mode=mybir.MatmulPerfMode.DoubleRow)
                    stc = rwp.tile([1, 2 * T1], F32, tag="stc", name="stc")
                    stb = st[:]
                    st0 = bass.AP(tensor=stb.tensor, offset=stb.offset,
                                  ap=[[stb.ap[0][0], 1], [1, 2 * T1]])
                    nc.scalar.copy(out=stc[:], in_=st0)

                    # ---- LN1 row math ----
                    s12 = rwp.tile([1, T1], F32, tag="s12", name="s12")
                    nc.vector.tensor_add(
                        s12[:],
                        _tap(stc, 0, [[T1, 2], [1, H1]]),
                        _tap(stc, H1, [[T1, 2], [1, H1]]))
                    mmsq = rwp.tile([1, T1], F32, tag="mmsq", name="mmsq")
                    nc.vector.tensor_scalar_mul(out=mmsq[:], in0=s12[:],
                                                scalar1=1.0 / (S * E))
                    m2t = rwp.tile([1, H1], F32, tag="m2t", name="m2t")
                    nc.vector.tensor_mul(m2t[:], mmsq[:, 0:H1], mmsq[:, 0:H1])
                    var = rwp.tile([1, H1], F32, tag="var", name="var")
                    nc.vector.tensor_sub(var[:], mmsq[:, H1:T1], m2t[:])
                    sd = rwp.tile([1, H1], F32, tag="sd", name="sd")
                    nc.scalar.activation(out=sd[:], in_=var[:], func=AT.Sqrt,
                                         bias=eps_sb[:])
                    rr = rwp.tile([1, H1], F32, tag="rr", name="rr")
                    nc.vector.reciprocal(out=rr[:], in_=sd[:])
                    rf2 = rvp.tile([1, T1], F16, tag="rf2", name="rf2")
                    nc.vector.tensor_copy(out=rf2[:, 0:H1], in_=rr[:])
                    nc.vector.tensor_copy(out=rf2[:, H1:T1], in_=rr[:])
                    r2f = rwp.tile([1, H1], F16, tag="r2f", name="r2f")
                    nc.vector.tensor_mul(r2f[:], rr[:], rr[:])
                    mf16 = rwp.tile([1, H1], F16, tag="mf16", name="mf16")
                    nc.vector.tensor_copy(out=mf16[:], in_=mmsq[:, 0:H1])
                    rmu = rwp.tile([1, H1], F32, tag="rmu", name="rmu")
                    nc.vector.tensor_mul(rmu[:], rr[:], mmsq[:, 0:H1])
                    nrmu2 = rvp.tile([1, T1], F16, tag="nrmu2", name="nrmu2")
                    nc.vector.tensor_scalar_mul(out=nrmu2[:, 0:H1],
                                                in0=rmu[:], scalar1=-1.0)
                    nc.vector.tensor_copy(out=nrmu2[:, H1:T1],
                                          in_=nrmu2[:, 0:H1])

                    # ---- dK = Wk @ dx  (fp8 DoubleRow, 3-set split;
                    # the Act drain descales by 1/(SK*SQ) so the later
                    # q*dK products need no extra scaling) ----
                    DR = mybir.MatmulPerfMode.DoubleRow
                    SETS = ((0, 0), (1, 0), (0, 1))[:QKV_SETS]
                    dkt = dkp.tile([P, E], F16, tag="dkt", name="dkt")
                    for half in range(2):
                        acc = pp.tile([P, 2 * T1], F32, tag="pj", name="kq")
                        nmm = 0
                        for mi in range(4):
                            mcol = (half * 4 + mi) * P
                            for wi, xi in SETS:
                                for kp in range(NCH // 2):
                                    nmm += 1
                                    MM(_tap(acc, mi * H1, [[1, H1]]),
                                       _tap(wk_sb[wi], 2 * kp * E + mcol,
                                            [[E, 2], [1, P]]),
                                       _tap(dxq[xi], 2 * kp * H1,
                                            [[H1, 2], [1, H1]]),
                                       start=(nmm == 1), stop=(nmm == 16 * len(SETS)),
                                       perf_mode=DR)
                        nc.scalar.activation(
                            out=dkt[:, half * 2 * T1:(half + 1) * 2 * T1],
                            in_=acc[:], func=AT.Copy,
                            scale=1.0 / (SK * SQ))

                    # ---- Q projection + q*dK products ----
                    pr = prp.tile([P, NCH * T1], F16, tag="pr", name="pr")
                    for q in range(4):
                        acc = pp.tile([P, 2 * T1], F32, tag="pj", name="qq")
                        nmm = 0
                        for mi in range(2):
                            mcol = (2 * q + mi) * P
                            for wi, xi in SETS:
                                for kp in range(NCH // 2):
                                    nmm += 1
                                    MM(_tap(acc, mi * T1, [[1, T1]]),
                                       _tap(wq_sb[wi], 2 * kp * E + mcol,
                                            [[E, 2], [1, P]]),
                                       _tap(xq[xi], 2 * kp * T1,
                                            [[T1, 2], [1, T1]]),
                                       start=(nmm == 1), stop=(nmm == 8 * len(SETS)),
                                       perf_mode=DR)
                        nc.vector.tensor_mul(
                            pr[:, q * 2 * T1:(q + 1) * 2 * T1],
                            acc[:],
                            _tap(dkt, q * T1, [[H1, 2], [0, 2], [1, H1]]))

                    # ---- scores ----
                    at = ps_a.tile([16, 4 * H1], F32, tag="at", name="at")
                    # at[:, 0:T1] = S~d ; at[:, T1:T1+H1] = Ad ;
                    # [T1+H1:..+3*H1] mu16/r216 written by rank-1 MMs below
                    for c in range(NCH):
                        MM(at[:, 0:T1], hsel_sb[:],
                           pr[:, c * T1:(c + 1) * T1],
                           start=(c == 0), stop=False)
                    for c in range(NCH):
                        MM(at[:, T1:T1 + H1],
                           cqsel_sb[:, c * 16:(c + 1) * 16],
                           dkt[:, c * H1:(c + 1) * H1],
                           start=False, stop=False)
                    MM(at[:, T1 + H1:T1 + 2 * H1], ones16_sb[:], mf16[:],
                       start=False, stop=True)
                    ps_r16 = ps_b.tile([16, 2 * H1], F32, tag="r16",
                                       name="r16")
                    MM(ps_r16[:, 0:H1], ones16_sb[:], r2f[:],
                       start=True, stop=False)
                    MM(ps_r16[:, H1:2 * H1], ones16_sb[:], rf2[:, 0:H1],
                       start=False, stop=True)
                    rpl = ps_c.tile([P, 2 * T1], F32, tag="rpl", name="rpl")
                    MM(rpl[:, 0:T1], onesR_sb[:], rf2[:],
                       start=True, stop=True)

                    # D = r^2*(S~d - m*Ad) ; p = sigmoid(D) ; p_r = p*r
                    adls = smp.tile([16, H1], F32, tag="adls",
                                    name="adls")
                    nc.scalar.copy(out=adls[:], in_=at[:, T1:T1 + H1])
                    t1 = smp.tile([16, H1], F32, tag="t1", name="t1")
                    nc.vector.tensor_mul(t1[:], adls[:],
                                         at[:, T1 + H1:T1 + 2 * H1])
                    d1 = smp.tile([16, T1], F32, tag="d1", name="d1")
                    nc.vector.tensor_sub(d1[:], at[:, 0:T1],
                                         _tap(t1, 0, [[0, 2], [1, H1]]))
                    d2 = smp.tile([16, T1], F32, tag="d2", name="d2")
                    nc.vector.tensor_mul(d2[:], d1[:],
                                         _tap(ps_r16, 0, [[0, 2], [1, H1]]))
                    psig = smp.tile([16, T1], F16, tag="psig", name="psig")
                    nc.scalar.activation(out=psig[:], in_=d2[:],
                                         func=AT.Sigmoid)
                    p_r = smp.tile([16, T1], F16, tag="p_r", name="p_r")
                    nc.vector.tensor_mul(p_r[:], psig[:],
                                         _tap(ps_r16, H1, [[0, 2], [1, H1]]))

                    # ---- V projection + dV / V1r + Z pieces ----
                    # px (p_r expanded over features) lives in rpl's second
                    # column half (one PSUM bank for both); it is emitted
                    # after the first V quarter so Z pieces overlap the
                    # remaining V matmuls.
                    dvt = dkp.tile([P, E], F16, tag="dvt", name="dvt")
                    v1r = dkp.tile([P, E], F16, tag="v1r", name="v1r")
                    z = zp.tile([P, NCH * T1], F16, tag="z", name="z")
                    for q in range(4):
                        acc = pp.tile([P, 2 * T1], F32, tag="pj", name="vq")
                        nmm = 0
                        for mi in range(2):
                            mcol = (2 * q + mi) * P
                            for wi, xi in SETS:
                                for kp in range(NCH // 2):
                                    nmm += 1
                                    MM(_tap(acc, mi * T1, [[1, T1]]),
                                       _tap(wv_sb[wi], 2 * kp * E + mcol,
                                            [[E, 2], [1, P]]),
                                       _tap(xq[xi], 2 * kp * T1,
                                            [[T1, 2], [1, T1]]),
                                       start=(nmm == 1), stop=(nmm == 8 * len(SETS)),
                                       perf_mode=DR)
                        if q == 0:
                            MM(rpl[:, T1:2 * T1], hselT_sb[:], p_r[:],
                               start=True, stop=True)
                        vs = dkp.tile([P, 2 * T1], F16, tag="vs", name="vs")
                        nc.scalar.activation(out=vs[:], in_=acc[:],
                                             func=AT.Copy, scale=1.0 / SV)
                        nc.vector.tensor_sub(
                            _tap(dvt, q * T1, [[H1, 2], [1, H1]]),
                            _tap(vs, 0, [[T1, 2], [1, H1]]),
                            _tap(vs, H1, [[T1, 2], [1, H1]]))
                        nc.vector.tensor_mul(
                            _tap(v1r, q * T1, [[H1, 2], [1, H1]]),
                            _tap(vs, H1, [[T1, 2], [1, H1]]),
                            _tap(rpl, 0, [[0, 2], [1, H1]]))
                        for s in range(2):
                            zqt = ztp.tile([P, T1], F16, tag="zqt",
                                           name="zqt")
                            nc.vector.scalar_tensor_tensor(
                                out=zqt[:],
                                in0=_tap(dvt, q * T1, [[H1, 2], [1, H1]]),
                                scalar=1.0,
                                in1=_tap(rpl, T1 + s * H1,
                                         [[0, 2], [1, H1]]),
                                op0=OP.mult, op1=OP.mult)
                            nc.vector.tensor_add(
                                _tap(z, q * 2 * T1 + s * H1,
                                     [[T1, 2], [1, H1]]),
                                zqt[:],
                                _tap(v1r, q * T1, [[H1, 2], [1, H1]]))

                    # ---- O-proj (+ rank-1 -r*m*co correction) ----
                    oacc = [pp.tile([P, 2 * T1], F32, tag="pj", name="oq")
                            for q in range(4)]
                    for c in range(NCH):
                        for q in range(4):
                            for mi in range(2):
                                m = 2 * q + mi
                                MM(oacc[q][:, mi * T1:(mi + 1) * T1],
                                   wo_sb[:, c * E + m * P:c * E + m * P + P],
                                   z[:, c * T1:(c + 1) * T1],
                                   start=(c == 0 and mi == 0), stop=False)
                    for q in range(4):
                        for mi in range(2):
                            m = 2 * q + mi
                            MM(oacc[q][:, mi * T1:(mi + 1) * T1],
                               cov_sb[:, m * P:(m + 1) * P], nrmu2[:],
                               start=False, stop=(mi == 1))

                    # ---- residual add -> out1 (fp16) ; spill ----
                    out1 = o1p.tile([P, NCH * T1], F16, tag="o1", name="o1")
                    for q in range(4):
                        nc.vector.tensor_add(
                            out1[:, q * 2 * T1:(q + 1) * 2 * T1],
                            oacc[q][:],
                            x8[:, q * 2 * T1:(q + 1) * 2 * T1])
                    for f2 in range(2):
                        dma(_ap(o1T.ap(), f2 * NCH * P * ROWS + t * H1,
                                [[ROWS, P], [P * ROWS, NCH], [1, H1]]),
                            _tap(out1, f2 * H1, [[T1, NCH], [1, H1]]))

                    # ---- LN2: square now, stats deferred ----
                    sq2 = sqp.tile([P, NCH * T1], F16, tag="sq2", name="sq2")
                    nc.scalar.activation(out=sq2[:], in_=out1[:],
                                         func=AT.Square)
                    emit_stats2(deferred[0])
                    deferred[0] = (out1, sq2, t)

                    if DEBUG and t == 0:
                        for nm, tl in (("stc", stc), ("rf2", rf2),
                                       ("dkt", dkt), ("d2", d2),
                                       ("p_r", p_r), ("z", z),
                                       ("nrmu2", nrmu2), ("v1r", v1r),
                                       ("rmu", rmu), ("mmsq", mmsq),
                                       ("rr", rr)):
                            dma(dbg[nm].ap(), tl[:])

                    prefetch(4)

                emit_stats2(deferred[0])
                prefetch(16)

            # =================== PASS 2 ===================
            with ExitStack() as p2:
                wff1 = p2.enter_context(tc.tile_pool(name="wff1", bufs=1))
                o1lp = p2.enter_context(tc.tile_pool(name="p2o1", bufs=3))
                osp = p2.enter_context(tc.tile_pool(name="p2os", bufs=2))
                hp = p2.enter_context(tc.tile_pool(name="p2h", bufs=2))
                yp = p2.enter_context(tc.tile_pool(name="p2y", bufs=2))
                fp = p2.enter_context(tc.tile_pool(name="p2f", bufs=2))
                ps_h = p2.enter_context(
                    tc.tile_pool(name="p2psh", bufs=4, space="PSUM"))
                ps_y = p2.enter_context(
                    tc.tile_pool(name="p2psy", bufs=2, space="PSUM"))
                ps_r = p2.enter_context(
                    tc.tile_pool(name="p2psr", bufs=2, space="PSUM"))

                w1_sb1 = [wff1.tile([P, NCH * HID], F8, tag=f"w1{i}",
                                    name=f"w1s1{i}") for i in range(2)]
                w2_sb1 = [wff1.tile([P, NHID * E], F8, tag=f"w2{i}",
                                    name=f"w2s1{i}") for i in range(2)]
                PFW = NCH * HID // 8
                pf2 = []
                for i in range(2):
                    pf2 += [(w1_sb1[i], w18[1][i], k) for k in range(8)]
                for i in range(2):
                    pf2 += [(w2_sb1[i], w28[1][i], k) for k in range(8)]

                def prefetch2(n):
                    for _ in range(n):
                        if not pf2:
                            return
                        t_sb, t_dr, k = pf2.pop(0)
                        dma(t_sb[:, k * PFW:(k + 1) * PFW],
                            _ap(t_dr.ap(), k * PFW,
                                [[NCH * HID, P], [1, PFW]]))

                tiles = [(f, u) for f in range(2) for u in range(NU)]

                def stage(i):
                    """Issue o1 load + r2 plane + r2-scale for tile i."""
                    f, u = tiles[i]
                    o1f = o1lp.tile([P, NCH * T2], F16, tag="o1f",
                                    name="o1f")
                    for ch in range(2):
                        nc.scalar.dma_start(
                            out=o1f[:, ch * 4 * T2:(ch + 1) * 4 * T2],
                            in_=_ap(o1T.ap(),
                                    (f * NCH + ch * 4) * P * ROWS + u * T2,
                                    [[ROWS, P], [P * ROWS, 4], [1, T2]]))
                    prefetch2(8)
                    r2pl = ps_r.tile([P, 2 * T2], F32, tag="r2pl",
                                     name="r2pl")
                    MM(r2pl[:, 0:T2], onesR_sb[:],
                       r2row[:, u * T2:(u + 1) * T2],
                       start=True, stop=False)
                    MM(r2pl[:, T2:2 * T2], onesR_sb[:],
                       nr2m2[:, u * T2:(u + 1) * T2],
                       start=False, stop=True)
                    osr = osp.tile([P, NCH * T2], F8, tag="osr",
                                   name="osr")
                    for q2 in range(4):
                        ost = osp.tile([P, 2 * T2], F16, tag="ost",
                                       name="ost")
                        nc.vector.scalar_tensor_tensor(
                            out=ost[:],
                            in0=_tap(o1f, q2 * 2 * T2, [[1, 2 * T2]]),
                            scalar=1.0,
                            in1=_tap(r2pl, 0, [[0, 2], [1, T2]]),
                            op0=OP.mult, op1=OP.mult)
                        nc.vector.tensor_add(
                            _tap(osr, q2 * 2 * T2, [[1, 2 * T2]]),
                            ost[:],
                            _tap(r2pl, T2, [[0, 2], [1, T2]]))
                    return o1f, osr

                staged = stage(0)
                for i in range(len(tiles)):
                    f, u = tiles[i]
                    o1f, osr = staged
                    if i + 1 < len(tiles):
                        staged = stage(i + 1)
                    w1s = w1_sb0 if f == 0 else w1_sb1
                    w2s = w2_sb0 if f == 0 else w2_sb1
                    DR = mybir.MatmulPerfMode.DoubleRow

                    h = hp.tile([P, NHID * T2], F8, tag="h", name="h")
                    for kc in range(NHID):
                        hacc = ps_h.tile([P, T2], F32, tag="hacc",
                                         name="hacc")
                        nmm = 0
                        for wi in range(W1_SETS):
                            for cp in range(NCH // 2):
                                nmm += 1
                                MM(hacc[:],
                                   _tap(w1s[wi], 2 * cp * HID + kc * P,
                                        [[HID, 2], [1, P]]),
                                   _tap(osr, 2 * cp * T2,
                                        [[T2, 2], [1, T2]]),
                                   start=(nmm == 1), stop=(nmm == 4 * W1_SETS),
                                   perf_mode=DR)
                        nc.scalar.activation(
                            out=h[:, kc * T2:(kc + 1) * T2], in_=hacc[:],
                            func=AT.Tanh, bias=b1_sb[f][:, kc:kc + 1],
                            scale=1.0 / SW1)

                    y = yp.tile([P, NCH * T2], F16, tag="y", name="y")
                    for mp_ in range(4):
                        ys = ps_y.tile([P, 2 * T2], F32, tag="ys",
                                       name="ys")
                        for mb in range(2):
                            m = 2 * mp_ + mb
                            nmm = 0
                            for wi in range(W2_SETS):
                                for kcp in range(NHID // 2):
                                    nmm += 1
                                    MM(_tap(ys, mb * T2, [[1, T2]]),
                                       _tap(w2s[wi], kcp * 2 * E + m * P,
                                            [[E, 2], [1, P]]),
                                       _tap(h, kcp * 2 * T2,
                                            [[T2, 2], [1, T2]]),
                                       start=(nmm == 1), stop=(nmm == 8 * W2_SETS),
                                       perf_mode=DR)
                        for mb in range(2):
                            m = 2 * mp_ + mb
                            nc.scalar.activation(
                                out=y[:, m * T2:(m + 1) * T2],
                                in_=ys[:, mb * T2:(mb + 1) * T2],
                                func=AT.Tanh, bias=b2_sb[f][:, m:m + 1],
                                scale=1.0 / SW2)

                    fin = fp.tile([P, NCH * T2], F32, tag="fin",
                                  name="fin")
                    for mp_ in range(4):
                        sl = slice(mp_ * 2 * T2, (mp_ + 1) * 2 * T2)
                        nc.vector.tensor_add(fin[:, sl], y[:, sl],
                                             o1f[:, sl])
                        dma(_ap(outT.ap(),
                                (f * NU + u) * NCH * P * T2
                                + mp_ * 2 * P * T2,
                                [[T2, P], [P * T2, 2], [1, T2]]),
                            fin[:, sl])
    return nc


_NC_CACHE = None


def _get_nc():
    global _NC_CACHE
    if _NC_CACHE is None:
        nc = bacc.Bacc("TRN2", target_bir_lowering=False, debug=False)
        build(nc)
        nc.compile()
        _NC_CACHE = nc
    return _NC_CACHE


def _prep_shared(inputs):
    f32, f16 = np.float32, np.float16
    f8 = ml_dtypes.float8_e4m3fn
    d = {}
    assert np.all(np.asarray(inputs["ln1_b"]) == 0), "ln1_b must be zero"
    assert np.all(np.asarray(inputs["ln2_b"]) == 0), "ln2_b must be zero"
    ln1w = np.asarray(inputs["ln1_w"], f32)
    ln2w = np.asarray(inputs["ln2_w"], f32)
    assert np.array_equal(ln1w[0], ln1w[1]), "ln1_w must match across s"

    def split8(W, s):
        """Scaled fp8 hi/lo split; returns (hi8, lo8, effective_W)."""
        Ws = W * s
        hi = Ws.astype(f8)
        lo = (Ws - hi.astype(f32)).astype(f8)
        eff = (hi.astype(f32) + lo.astype(f32)) / s
        return hi, lo, eff

    def wbig(w):
        # lhsT layout: arr[p, c*E + m*128 + mm] = w[m*128+mm, c*128+p]
        Eo, Ei = w.shape
        return np.ascontiguousarray(
            w.reshape(Eo // P, P, Ei // P, P).transpose(3, 2, 0, 1)
            .reshape(P, Ei // P * Eo))

    Wq = np.asarray(inputs["Wq"], f32) * (ln1w[0] / 8.0)[None, :]
    Wk = np.asarray(inputs["Wk"], f32) * ln1w[0][None, :]
    Wv = np.asarray(inputs["Wv"], f32) * ln1w[0][None, :]
    Wo = np.asarray(inputs["Wo"], f32)

    qhi, qlo, qeff = split8(Wq, SQ)
    khi, klo, _ = split8(Wk, SK)
    vhi, vlo, veff = split8(Wv, SV)
    for i, a in enumerate((qhi, qlo)):
        d[f"wq8{i}"] = wbig(a)
    for i, a in enumerate((khi, klo)):
        d[f"wk8{i}"] = wbig(a)
    for i, a in enumerate((vhi, vlo)):
        d[f"wv8{i}"] = wbig(a)
    Woh = Wo.astype(f16).astype(f32)
    d["wobig"] = wbig(Wo).astype(f16)

    cqv = qeff.sum(1)                   # [E] effective row sums
    cvv = veff.sum(1)
    cov = Woh @ cvv
    d["cov"] = cov.reshape(1, E).astype(f16)

    cqsel = np.zeros((P, NCH * 16), f32)
    for c in range(NCH):
        dd = np.arange(P)
        cqsel[dd, c * 16 + (dd % 16)] = cqv[c * P + dd] * SQ
    d["cqsel"] = cqsel.astype(f16)
    hs = np.zeros((P, 16), f32)
    hs[np.arange(P), np.arange(P) % 16] = 1.0
    d["hsel"] = hs.astype(f16)
    d["hselT"] = np.ascontiguousarray(hs.T).astype(f16)
    d["onesP"] = np.ones((P, 1), f16)
    d["ones8"] = np.ones((P, 32), ml_dtypes.float8_e4m3fn)
    d["onesR"] = np.ones((1, P), f16)
    d["ones16"] = np.ones((1, 16), f16)

    for f in range(2):
        W1 = np.asarray(inputs["f1w1" if f == 0 else "f2w1"], f32) \
            * ln2w[f][None, :]
        W2 = np.asarray(inputs["f1w2" if f == 0 else "f2w2"], f32)
        B1 = np.asarray(inputs["f1b1" if f == 0 else "f2b1"], f32)
        B2 = np.asarray(inputs["f1b2" if f == 0 else "f2b2"], f32)
        w1hi, w1lo, _ = split8(W1, SW1)
        w2hi, w2lo, _ = split8(W2, SW2)
        # w1big: arr[p, c*HID + kc*128 + mm] = W1[kc*128+mm, c*128+p]
        for i, a in enumerate((w1hi, w1lo)):
            d[f"w18{f}{i}"] = np.ascontiguousarray(
                a.reshape(NHID, P, NCH, P).transpose(3, 2, 0, 1)
                .reshape(P, NCH * HID))
        # w2big: arr[p, kc*E + m*128 + mm] = W2[m*128+mm, kc*128+p]
        for i, a in enumerate((w2hi, w2lo)):
            d[f"w28{f}{i}"] = np.ascontiguousarray(
                a.reshape(NCH, P, NHID, P).transpose(3, 2, 0, 1)
                .reshape(P, NHID * E))
        d[f"b1f{f}"] = np.ascontiguousarray(
            B1.reshape(NHID, P).T).astype(f32)
        d[f"b2f{f}"] = np.ascontiguousarray(
            B2.reshape(NCH, P).T).astype(f32)
    return d


def _prep_core(inputs, core):
    f32 = np.float32
    f8 = ml_dtypes.float8_e4m3fn
    rows0 = core * ROWS
    chunk = np.asarray(inputs["input"][rows0:rows0 + ROWS], f32)
    # [t, j, s, c, p] -> [t, c, p, s, j]
    arr = chunk.reshape(NT, H1, S, NCH, P).transpose(0, 3, 4, 2, 1)
    xprep = np.ascontiguousarray(arr.reshape(NT * NCH * P, T1)) \
        .astype(np.float16)
    xf = xprep.astype(f32)
    xhi = xf.astype(f8)
    xlo = (xf - xhi.astype(f32)).astype(f8)
    # dx[t, c, p, j] = x[.., s=0] - x[.., s=1]
    dx = (arr[:, :, :, 0, :] - arr[:, :, :, 1, :]).reshape(NT * NCH * P, H1)
    dxhi = dx.astype(f8)
    dxlo = (dx - dxhi.astype(f32)).astype(f8)
    return {"xprep": xprep, "xhi8": xhi, "xlo8": xlo,
            "dxhi8": dxhi, "dxlo8": dxlo}


def _decode_out(outT):
    # outT rows: (((f*NU + u)*NCH + c)*P + p), cols j
    arr = outT.reshape(2, NU, NCH, P, T2)
    out = np.empty((ROWS, S, E), np.float32)
    for f in range(2):
        for u in range(NU):
            # arr[f,u,c,p,j] -> out[u*T2+j, f, c*128+p]
            out[u * T2:(u + 1) * T2, f, :] = \
                arr[f, u].transpose(2, 0, 1).reshape(T2, E)
    return out


def kernel(**inputs):
    nc = _get_nc()
    shared = _prep_shared(inputs)
    in_maps = []
    for core in range(NCORES):
        m = dict(shared)
        m.update(_prep_core(inputs, core))
        in_maps.append(m)
    res = bass_utils.run_bass_kernel_spmd(nc, in_maps,
                                          core_ids=list(range(NCORES)))
    out = np.empty((B, S, E), np.float32)
    for core in range(NCORES):
        out[core * ROWS:(core + 1) * ROWS] = _decode_out(
            res.results[core]["outT"])
    return out

